# revision 8
# baseline (speedup 1.0000x reference)
"""SwitchBack global-quantized MLP on 8 TRN2 NeuronCores.

Strategy: data-parallel over the 8192 token rows (1024 rows/core, zero
collectives).  Weights are globally int8-quantized on the host (static
prep, numerically identical to the reference: np.round == round-half-even)
and shipped pre-transposed in bf16 (int8 values are exact in bf16; all
matmul products accumulate exactly in fp32 PSUM).  Activations are
quantized on-device: rowwise absmax -> reciprocal -> magic-number
round-to-nearest-even -> bf16, DMA-xbar transpose to put the contraction
dim on partitions.
"""

import numpy as np
import ml_dtypes

import concourse.bass as bass
import concourse.mybir as mybir
import concourse.tile as tile
from concourse import bacc
from concourse.bass_utils import run_bass_kernel_spmd

Q = 127.0
MAGIC = 12582912.0  # 1.5 * 2**23: (v + MAGIC) - MAGIC == RNE-round(v) for |v| <= 2**22
P = 128
FD = 512  # matmul moving free dim == one PSUM bank of fp32

F32 = mybir.dt.float32
BF16 = mybir.dt.bfloat16


def build_program(NR, D, H, c1, c2, n_cores=8, gelu_mode="sigmoid"):
    """One-core SPMD program: NR token rows, x[NR,D] @ W1qT[D,H] -> gelu ->
    requant -> @ W2qT[H,D] -> out[NR,D].  c1/c2 = sW/(Q*Q) dequant consts."""
    MT, KD, NH, KH, ND = NR // P, D // P, H // FD, H // P, D // FD
    AF = mybir.ActivationFunctionType
    OP = mybir.AluOpType
    GELU_A = 0.044715
    GELU_2C = float(2.0 * np.sqrt(2.0 / np.pi))

    nc = bacc.Bacc("TRN2", target_bir_lowering=False, debug=False,
                   num_devices=n_cores)
    x_d = nc.dram_tensor("x", [NR, D], F32, kind="ExternalInput")
    w1_d = nc.dram_tensor("w1qt", [D, H], BF16, kind="ExternalInput")
    w2_d = nc.dram_tensor("w2qt", [H, D], BF16, kind="ExternalInput")
    b1_d = nc.dram_tensor("b1r", [P, H], F32, kind="ExternalInput")
    b2_d = nc.dram_tensor("b2r", [P, D], F32, kind="ExternalInput")
    out_d = nc.dram_tensor("out", [NR, D], F32, kind="ExternalOutput")
    x2_d = nc.dram_tensor("x2f", [NR, H], F32)  # gelu output, pre-requant

    with tile.TileContext(nc) as tc:
        with (
            tc.tile_pool(name="glob", bufs=1) as gp,
            tc.tile_pool(name="psum", bufs=8, space="PSUM") as pp,
        ):
            # Persistent per-row scalars, one column per m-tile.
            ds1 = gp.tile([P, MT], F32, tag="ds1", name="ds1")
            rmax = gp.tile([P, MT], F32, tag="rmax", name="rmax")
            s2 = gp.tile([P, MT], F32, tag="s2", name="s2")
            ds2 = gp.tile([P, MT], F32, tag="ds2", name="ds2")
            b2_sb = gp.tile([P, D], F32, tag="b2", name="b2_sb")
            nc.sync.dma_start(out=b2_sb, in_=b2_d[:, :])
            nc.vector.memset(rmax, 0.0)

            # ---------------- phase A: quantize x, matmul1, gelu ----------
            with (
                tc.tile_pool(name="pa", bufs=1) as pa,
                tc.tile_pool(name="wa", bufs=16) as wa,
                tc.tile_pool(name="ea", bufs=4) as ea,
            ):
                b1_sb = pa.tile([P, H], F32, tag="b1", name="b1_sb")
                nc.sync.dma_start(out=b1_sb, in_=b1_d[:, :])
                x1T = [pa.tile([P, NR], BF16, tag=f"x1T{k}", name=f"x1T{k}")
                       for k in range(KD)]
                for m in range(MT):
                    xt = ea.tile([P, D], F32, tag="xt", name=f"xt{m}", bufs=2)
                    nc.sync.dma_start(out=xt, in_=x_d[m * P:(m + 1) * P, :])
                    am = ea.tile([P, 1], F32, tag="am", name=f"am{m}")
                    nc.vector.tensor_reduce(am, xt, axis=mybir.AxisListType.X,
                                            op=OP.max, apply_absolute_value=True)
                    rr = ea.tile([P, 1], F32, tag="rr", name=f"rr{m}")
                    nc.vector.reciprocal(rr, am)
                    s1m = ea.tile([P, 1], F32, tag="s1m", name=f"s1m{m}")
                    nc.vector.tensor_scalar_mul(s1m, rr, Q)
                    nc.vector.tensor_scalar_mul(ds1[:, m:m + 1], am, c1)
                    tq = ea.tile([P, D], F32, tag="tq", name=f"tq{m}", bufs=2)
                    nc.scalar.activation(tq, xt, AF.Copy, bias=MAGIC, scale=s1m)
                    x1q = ea.tile([P, D], BF16, tag="x1q", name=f"x1q{m}", bufs=3)
                    nc.vector.tensor_scalar_sub(x1q, tq, MAGIC)
                    for k in range(KD):
                        nc.sync.dma_start_transpose(
                            out=x1T[k][:, m * P:(m + 1) * P],
                            in_=x1q[:, k * P:(k + 1) * P])

                for n in range(NH):
                    w1t = []
                    for k in range(KD):
                        w = wa.tile([P, FD], BF16, tag="w1t", name=f"w1t{n}_{k}")
                        nc.sync.dma_start(
                            out=w, in_=w1_d[k * P:(k + 1) * P,
                                            n * FD:(n + 1) * FD])
                        w1t.append(w)
                    pss = [pp.tile([P, FD], F32, tag="ps", name=f"psA{n}_{m}")
                           for m in range(MT)]
                    for k in range(KD):
                        for m in range(MT):
                            nc.tensor.matmul(pss[m],
                                             x1T[k][:, m * P:(m + 1) * P],
                                             w1t[k],
                                             start=(k == 0), stop=(k == KD - 1))
                    for m in range(MT):
                        t1 = ea.tile([P, FD], F32, tag="t1", name=f"t1_{n}_{m}")
                        nc.vector.scalar_tensor_tensor(
                            t1, pss[m], ds1[:, m:m + 1],
                            b1_sb[:, n * FD:(n + 1) * FD],
                            op0=OP.mult, op1=OP.add)
                        g = ea.tile([P, FD], F32, tag="g", name=f"g{n}_{m}")
                        if gelu_mode == "lut":
                            nc.scalar.activation(g, t1, AF.Gelu_apprx_tanh)
                        else:
                            # gelu_tanh(x) = x * sigmoid(2c * x * (1 + a*x^2))
                            sq = ea.tile([P, FD], F32, tag="sq", name=f"sq{n}_{m}")
                            nc.vector.tensor_tensor(sq, t1, t1, OP.mult)
                            nc.vector.tensor_scalar(sq, sq, GELU_A, 1.0,
                                                    op0=OP.mult, op1=OP.add)
                            nc.vector.tensor_tensor(sq, t1, sq, OP.mult)
                            sg = ea.tile([P, FD], F32, tag="sg", name=f"sg{n}_{m}")
                            nc.scalar.activation(sg, sq, AF.Sigmoid,
                                                 scale=GELU_2C)
                            nc.vector.tensor_tensor(g, t1, sg, OP.mult)
                        pm = ea.tile([P, 1], F32, tag="pm", name=f"pm{n}_{m}")
                        nc.vector.tensor_reduce(pm, g, axis=mybir.AxisListType.X,
                                                op=OP.max,
                                                apply_absolute_value=True)
                        nc.vector.tensor_tensor(rmax[:, m:m + 1],
                                                rmax[:, m:m + 1], pm, OP.max)
                        nc.sync.dma_start(
                            out=x2_d[m * P:(m + 1) * P, n * FD:(n + 1) * FD],
                            in_=g)

            # ---------------- phase B: requantize X2; phase C: matmul2 ----
            with (
                tc.tile_pool(name="pc", bufs=1) as pc,
                tc.tile_pool(name="wc", bufs=16) as wc,
                tc.tile_pool(name="eb", bufs=4) as eb,
            ):
                rr2 = eb.tile([P, MT], F32, tag="rr2", name="rr2")
                nc.vector.reciprocal(rr2, rmax)
                nc.vector.tensor_scalar_mul(s2, rr2, Q)
                nc.vector.tensor_scalar_mul(ds2, rmax, c2)

                x2T = [pc.tile([P, NR], BF16, tag=f"x2T{k}", name=f"x2T{k}")
                       for k in range(KH)]
                for n in range(NH):
                    for m in range(MT):
                        xt2 = eb.tile([P, FD], F32, tag="xt2", name=f"xt2_{n}_{m}")
                        nc.sync.dma_start(
                            out=xt2, in_=x2_d[m * P:(m + 1) * P,
                                              n * FD:(n + 1) * FD])
                        tq2 = eb.tile([P, FD], F32, tag="tq2", name=f"tq2_{n}_{m}")
                        nc.scalar.activation(tq2, xt2, AF.Copy, bias=MAGIC,
                                             scale=s2[:, m:m + 1])
                        q2 = eb.tile([P, FD], BF16, tag="q2", name=f"q2_{n}_{m}")
                        nc.vector.tensor_scalar_sub(q2, tq2, MAGIC)
                        for j in range(FD // P):
                            k = n * (FD // P) + j
                            nc.sync.dma_start_transpose(
                                out=x2T[k][:, m * P:(m + 1) * P],
                                in_=q2[:, j * P:(j + 1) * P])

                for d in range(ND):
                    pss2 = [pp.tile([P, FD], F32, tag="ps", name=f"psC{d}_{m}")
                            for m in range(MT)]
                    for k in range(KH):
                        w2t = wc.tile([P, FD], BF16, tag="w2t",
                                      name=f"w2t{d}_{k}")
                        nc.sync.dma_start(
                            out=w2t, in_=w2_d[k * P:(k + 1) * P,
                                              d * FD:(d + 1) * FD])
                        for m in range(MT):
                            nc.tensor.matmul(pss2[m],
                                             x2T[k][:, m * P:(m + 1) * P],
                                             w2t,
                                             start=(k == 0), stop=(k == KH - 1))
                    for m in range(MT):
                        o = eb.tile([P, FD], F32, tag="o", name=f"o{d}_{m}")
                        nc.vector.scalar_tensor_tensor(
                            o, pss2[m], ds2[:, m:m + 1],
                            b2_sb[:, d * FD:(d + 1) * FD],
                            op0=OP.mult, op1=OP.add)
                        nc.sync.dma_start(
                            out=out_d[m * P:(m + 1) * P, d * FD:(d + 1) * FD],
                            in_=o)
    nc.compile()
    return nc


def _host_prep(x, W1, B1, W2, B2, n_cores=8):
    B, S, D = x.shape
    H = W1.shape[0]
    N = B * S
    NR = N // n_cores
    X = np.ascontiguousarray(x.reshape(N, D))

    def quant_global_T(w):
        # match jnp: absmax in f32, scale = f32(127)/absmax, round-half-even
        am = np.float32(np.max(np.abs(w)))
        scale = np.float32(Q) / am
        q = np.round(w.astype(np.float32) * scale)
        return np.ascontiguousarray(q.T).astype(ml_dtypes.bfloat16), am

    W1qT, sW1 = quant_global_T(W1)  # [D, H]
    W2qT, sW2 = quant_global_T(W2)  # [H, D]
    c1 = float(sW1) / (Q * Q)
    c2 = float(sW2) / (Q * Q)
    b1r = np.ascontiguousarray(np.broadcast_to(B1.astype(np.float32), (P, H)))
    b2r = np.ascontiguousarray(np.broadcast_to(B2.astype(np.float32), (P, D)))

    in_maps = [
        {"x": X[i * NR:(i + 1) * NR], "w1qt": W1qT, "w2qt": W2qT,
         "b1r": b1r, "b2r": b2r}
        for i in range(n_cores)
    ]
    return in_maps, NR, D, H, c1, c2


def _run_sharded(nc, in_maps, n_cores, bench_iters=0):
    """Mirror bass2jax.run_bass_via_pjrt's multi-core path, with an optional
    steady-state timing loop over device-resident inputs."""
    import time

    import jax
    from jax.sharding import Mesh, NamedSharding, PartitionSpec
    from jax.experimental.shard_map import shard_map
    import concourse.mybir as mybir_
    from concourse import bass2jax

    bass2jax.install_neuronx_cc_hook()

    partition_name = (nc.partition_id_tensor.name
                      if nc.partition_id_tensor else None)
    in_names, out_names, out_avals, zero_outs = [], [], [], []
    for alloc in nc.m.functions[0].allocations:
        if not isinstance(alloc, mybir_.MemoryLocationSet):
            continue
        name = alloc.memorylocations[0].name
        if alloc.kind == "ExternalInput":
            if name != partition_name:
                in_names.append(name)
        elif alloc.kind == "ExternalOutput":
            out_names.append(name)
            shape = tuple(alloc.tensor_shape)
            dtype = mybir_.dt.np(alloc.dtype)
            out_avals.append(jax.core.ShapedArray(shape, dtype))
            zero_outs.append(np.zeros(shape, dtype))
    n_params = len(in_names)
    n_outs = len(out_avals)
    in_names = in_names + out_names
    if partition_name is not None:
        in_names.append(partition_name)
    donate = tuple(range(n_params, n_params + n_outs))

    def _body(*args):
        operands = list(args)
        if partition_name is not None:
            operands.append(bass2jax.partition_id_tensor())
        return tuple(bass2jax._bass_exec_p.bind(
            *operands,
            out_avals=tuple(out_avals),
            in_names=tuple(in_names),
            out_names=tuple(out_names),
            lowering_input_output_aliases=(),
            sim_require_finite=True,
            sim_require_nnan=True,
            nc=nc,
        ))

    devices = jax.devices()[:n_cores]
    mesh = Mesh(np.asarray(devices), ("core",))
    spec = NamedSharding(mesh, PartitionSpec("core"))
    sharded = jax.jit(
        shard_map(_body, mesh=mesh,
                  in_specs=(PartitionSpec("core"),) * (n_params + n_outs),
                  out_specs=(PartitionSpec("core"),) * n_outs,
                  check_rep=False),
        donate_argnums=donate, keep_unused=True)

    concat_in = [
        np.concatenate([np.asarray(in_maps[c][name]) for c in range(n_cores)],
                       axis=0)
        for name in in_names[:n_params]
    ]
    dev_in = [jax.device_put(a, spec) for a in concat_in]
    big_zeros = [np.zeros((n_cores * z.shape[0], *z.shape[1:]), z.dtype)
                 for z in zero_outs]

    def fresh_zeros():
        return [jax.device_put(z, spec) for z in big_zeros]

    out_arrs = sharded(*dev_in, *fresh_zeros())
    jax.block_until_ready(out_arrs)

    per_iter_s = None
    if bench_iters > 1:
        zero_sets = [fresh_zeros() for _ in range(bench_iters)]
        jax.block_until_ready(zero_sets)
        t0 = time.perf_counter()
        last = None
        for k in range(bench_iters):
            last = sharded(*dev_in, *zero_sets[k])
        jax.block_until_ready(last)
        per_iter_s = (time.perf_counter() - t0) / bench_iters

    results = [
        {name: np.asarray(out_arrs[i]).reshape(n_cores, *out_avals[i].shape)[c]
         for i, name in enumerate(out_names)}
        for c in range(n_cores)
    ]
    return results, per_iter_s


def kernel_with_results(x, W1, B1, W2, B2, bench_iters=0):
    n_cores = 8
    in_maps, NR, D, H, c1, c2 = _host_prep(x, W1, B1, W2, B2, n_cores)
    nc = build_program(NR, D, H, c1, c2, n_cores)
    results, per_iter_s = _run_sharded(nc, in_maps, n_cores, bench_iters)
    out = np.concatenate([r["out"] for r in results], axis=0)
    return out.reshape(x.shape).astype(np.float32), per_iter_s


def kernel(x, W1, B1, W2, B2):
    return kernel_with_results(x, W1, B1, W2, B2)[0]
